# revision 21
# baseline (speedup 1.0000x reference)
"""GroupedQueryAttention Trainium2 kernel (bf16 pipeline).

Problem shapes (hardcoded): x [2, 2048, 1024], H=16 heads, G=4 kv-groups,
head_dim=64.  out = softmax((xWq)(xWk)^T / 8) (xWv) Wo + biases.

Sharding: 8 cores, core d = (b, j) with b = d // 4, j = d % 4.
Each core computes the full attention output for batch b, query rows
[512j, 512j+512), all 16 heads — output rows are complete per core, so the
host-side gather is a pure concat (no reduction).
K/V are computed per-core for the whole batch (cheap 4x duplication).
The token axis of x^T is rolled per-core so queries are always columns
0:512 (attention is permutation-invariant over keys), keeping the SPMD
program identical across cores.

All matmul operands are bf16 (host-cast), PSUM accumulation fp32.  The
f32r path used previously lowers to fp32_mode=HIGH matmuls on HW (~3x
slower) and defeats fast-weight-load; bf16 fixes both and halves DMA.

On-chip dataflow (per core), "feature-on-partition" layout:
  x^T is pre-transposed + bf16-cast on host and DMA'd directly.
  K^T[dg,n]  = Wk^T x_b^T   (PSUM accum over c-chunks, + bias on DVE)
  V[n,dg]    = x_b Wv       (natural layout, + ones column for softmax denom)
  Q^T[d,nq]  = Wq^T x_q^T
  Attention runs per head-PAIR (2t, 2t+1) — both heads of a pair share
  Q^T chunk t (rows 0:64 / 64:128) and land in oT chunk t; their kv
  groups (2t%4, (2t+1)%4) share a kT gt-chunk.  Per k-chunk:
    S^T[k, 2, nq] = K Q^T   (two matmuls into one PSUM tile)
    P^T = exp(S^T / 8)      (one ScalarE activation per 1024-wide batch,
                             scale folded; logits are O(1), no max sub)
    O^T[65,nq] += [V|1]^T P^T  (per head; row 64 = denominator)
  Program order emits exp(kc); scores(kc+1); AV(kc) so the in-order
  TensorE queue hides AV's wait-for-exp behind next-chunk scores.
  normalize: per pair, reciprocal_approx_fast on the two denom rows,
  broadcast to 128 partitions via a K=2 one-hot matmul, one DVE mul.
  y[nq, c]   = O^T^T Wo     (accumulate over c-chunks) + bo
"""

import ml_dtypes
import numpy as np

import concourse.bacc as bacc
import concourse.mybir as mybir
import concourse.tile as tile
from concourse.bass_utils import run_bass_kernel_spmd

# ---- problem constants (hardcoded per contract) ----
B, N, C = 2, 2048, 1024
H, G, HD = 16, 4, 64
DG = G * HD            # 256
NCORES = 8
SPLIT = NCORES // B    # 4 query splits per batch
NQ = N // SPLIT        # 512 query rows per core
P = 128
CT = C // P            # 8 c-chunks
KC = N // P            # 16 k-chunks
SCALE = HD ** -0.5

F32 = mybir.dt.float32
BF16 = mybir.dt.bfloat16
NPBF = ml_dtypes.bfloat16

_CACHE = {}


def _build():
    nc = bacc.Bacc(None, target_bir_lowering=False)

    xbT = nc.declare_dram_parameter("xbT", [C, N], BF16, isOutput=False)
    Wq = nc.declare_dram_parameter("Wq", [C, C], BF16, isOutput=False)
    Wk = nc.declare_dram_parameter("Wk", [C, DG], BF16, isOutput=False)
    Wv = nc.declare_dram_parameter("Wv", [C, DG], BF16, isOutput=False)
    Wo = nc.declare_dram_parameter("Wo", [C, C], BF16, isOutput=False)
    bq = nc.declare_dram_parameter("bq", [C], BF16, isOutput=False)
    bk = nc.declare_dram_parameter("bk", [DG], BF16, isOutput=False)
    bv = nc.declare_dram_parameter("bv", [DG], BF16, isOutput=False)
    bo = nc.declare_dram_parameter("bo", [C], BF16, isOutput=False)
    y = nc.declare_dram_parameter("y", [NQ, C], F32, isOutput=True)

    with tile.TileContext(nc) as tc:
        with tc.tile_pool(name="main", bufs=1) as main:
            qT = main.tile([P, CT, NQ], BF16)         # Q^T  d-chunk x q
            kT = main.tile([P, 2, N], BF16)           # K^T  dg-chunk x k
            vA = main.tile([P, KC, G, HD + 1], BF16)  # V + ones col, per k-chunk
            oT = main.tile([P, CT, NQ], BF16)         # O^T (unnorm -> normed)
            wo = main.tile([P, CT, C], BF16)
            eh = main.tile([1, 2, P], BF16)           # one-hot head->rows map
            ones1 = main.tile([1, 512], BF16)         # ones row (lhs or rhs)
            bqr = main.tile([1, C], BF16)             # bias rows (natural)
            bkr = main.tile([1, DG], BF16)
            bvr = main.tile([1, DG], BF16)
            bor = main.tile([1, C], BF16)

            # ---------------- phase A+B: load + projections ----------------
            # DMA priority: xbT+wk on the sync queue (K streams against
            # their arrival); wv/wq via scalar, wo via vector, consts and
            # bias rows via gpsimd — parallel issue queues, so the first
            # xbT chunk lands ~1us in.
            with tc.tile_pool(name="proj", bufs=1) as proj:
                xbTs = proj.tile([P, CT, N], BF16)
                wq = proj.tile([P, CT, C], BF16)
                wk = proj.tile([P, CT, DG], BF16)
                wv = proj.tile([P, CT, DG], BF16)
                for t in range(CT):
                    nc.sync.dma_start(out=xbTs[:, t, :], in_=xbT[t * P:(t + 1) * P, :])
                    nc.sync.dma_start(out=wk[:, t, :], in_=Wk[t * P:(t + 1) * P, :])
                for t in range(CT):
                    nc.scalar.dma_start(out=wv[:, t, :], in_=Wv[t * P:(t + 1) * P, :])
                for t in range(CT):
                    nc.scalar.dma_start(out=wq[:, t, :], in_=Wq[t * P:(t + 1) * P, :])
                # constants DMA'd from NEFF-embedded data (before wo on the
                # gpsimd queue — needed much earlier)
                e_np = np.zeros((1, 2, P), NPBF)
                e_np[0, 0, 0:HD] = 1.0   # even head of pair -> rows 0..63
                e_np[0, 1, HD:P] = 1.0   # odd head of pair -> rows 64..127
                nc.gpsimd.dma_start(out=eh[:], in_=nc.inline_tensor(e_np, "ehot")[:])
                nc.gpsimd.dma_start(
                    out=ones1[:],
                    in_=nc.inline_tensor(np.ones((1, 512), NPBF), "ones1")[:])
                vcol_np = np.ones((P, KC * G), NPBF)
                nc.gpsimd.dma_start(
                    out=vA[:, :, :, HD:HD + 1],
                    in_=nc.inline_tensor(vcol_np, "vcol")[:]
                    .rearrange("p (k g o) -> p k g o", g=G, o=1))
                nc.gpsimd.dma_start(out=bqr[:], in_=bq.rearrange("(o d) -> o d", o=1))
                nc.gpsimd.dma_start(out=bkr[:], in_=bk.rearrange("(o d) -> o d", o=1))
                nc.gpsimd.dma_start(out=bvr[:], in_=bv.rearrange("(o d) -> o d", o=1))
                nc.gpsimd.dma_start(out=bor[:], in_=bo.rearrange("(o d) -> o d", o=1))
                for t in range(CT):
                    nc.gpsimd.dma_start(out=wo[:, t, :], in_=Wo[t * P:(t + 1) * P, :])

                # pre-warm the exp table set while DMAs stream
                warm = proj.tile([1, 2], F32)
                nc.scalar.activation(warm[:], ones1[0:1, 0:2],
                                     mybir.ActivationFunctionType.Exp)

                with nc.allow_low_precision(reason="bf16 staging of projections"):
                    # K^T, t-streamed against DMA arrival: all 8 output
                    # blocks accumulate in parallel (8 PSUM banks) so the
                    # first matmul only needs chunk 0 of xbT/wk.  Bias is
                    # added via a [1,w] x [1,512] broadcast matmul (bias
                    # values as weights, ones as the moving operand).
                    with tc.tile_pool(name="pk8", bufs=1, space="PSUM") as pk8:
                        pks = [pk8.tile([P, 512], F32, tag=f"pk{b}",
                                        name=f"pk{b}")
                               for b in range(8)]
                        for t in range(CT):
                            for gt in range(2):
                                for nf in range(N // 512):
                                    nc.tensor.matmul(
                                        pks[gt * 4 + nf],
                                        wk[:, t, gt * P:(gt + 1) * P],
                                        xbTs[:, t, nf * 512:(nf + 1) * 512],
                                        start=(t == 0), stop=False)
                        for gt in range(2):
                            for nf in range(N // 512):
                                nc.tensor.matmul(
                                    pks[gt * 4 + nf],
                                    bkr[0:1, gt * P:(gt + 1) * P],
                                    ones1[0:1, :], start=False, stop=True)
                                nc.vector.tensor_copy(
                                    kT[:, gt, nf * 512:(nf + 1) * 512],
                                    pks[gt * 4 + nf])

                    # V-low (kv groups 0,1) + bias; V-high runs as filler
                    # inside the ScalarE-bound attention loop.
                    with tc.tile_pool(name="pp", bufs=2, space="PSUM") as pp:
                        for kc in range(KC):
                            pv = pp.tile([P, P], F32, tag="pv")
                            for t in range(CT):
                                nc.tensor.matmul(
                                    pv[:], xbTs[:, t, kc * P:(kc + 1) * P],
                                    wv[:, t, 0:P], start=(t == 0), stop=False)
                            nc.tensor.matmul(pv[:], ones1[0:1, 0:P],
                                             bvr[0:1, 0:P],
                                             start=False, stop=True)
                            nc.vector.tensor_copy(
                                vA[:, kc, 0:2, 0:HD],
                                pv[:].rearrange("p (g d) -> p g d", g=2))

                        # Q^T chunk 0 only; chunks 1..7 run as attention filler
                        pq = pp.tile([P, NQ], F32, tag="pv")
                        for t in range(CT):
                            nc.tensor.matmul(
                                pq[:], wq[:, t, 0:P],
                                xbTs[:, t, 0:NQ], start=(t == 0), stop=False)
                        nc.tensor.matmul(pq[:], bqr[0:1, 0:P], ones1[0:1, :],
                                         start=False, stop=True)
                        nc.vector.tensor_copy(qT[:, 0, :], pq[:])

                # ---- phase C: attention (per head-pair) with fillers ----
                # Pair order: even pairs first (heads in kv groups 0,1 —
                # only V-low needed), then odd pairs (groups 2,3).  V-high
                # columns and the next pair's Q^T chunk are emitted as
                # TensorE filler inside each ScalarE-bound kc loop.
                pair_order = [0, 2, 4, 6, 1, 3, 5, 7]
                with tc.tile_pool(name="pt", bufs=3) as ptp, \
                     tc.tile_pool(name="rd", bufs=2) as rdp, \
                     tc.tile_pool(name="ps", bufs=2, space="PSUM") as psp, \
                     tc.tile_pool(name="po", bufs=1, space="PSUM") as pop, \
                     tc.tile_pool(name="pb", bufs=1, space="PSUM") as pbp, \
                     tc.tile_pool(name="fl", bufs=1, space="PSUM") as flp:
                    for pi, t in enumerate(pair_order):
                        hA, hB = 2 * t, 2 * t + 1
                        gA, gB = hA % G, hB % G
                        gtA, grA = gA // 2, (gA % 2) * HD
                        gtB, grB = gB // 2, (gB % 2) * HD
                        q_A = qT[0:HD, t, :]
                        q_B = qT[HD:P, t, :]
                        poA = pop.tile([HD + 1, NQ], F32, tag="poA")
                        poB = pop.tile([HD + 1, NQ], F32, tag="poB")
                        state = {"fl": None}

                        def scores(kc):
                            ps = psp.tile([P, 2, NQ], F32)
                            nc.tensor.matmul(
                                ps[:, 0, :],
                                kT[grA:grA + HD, gtA, kc * P:(kc + 1) * P],
                                q_A, start=True, stop=True)
                            nc.tensor.matmul(
                                ps[:, 1, :],
                                kT[grB:grB + HD, gtB, kc * P:(kc + 1) * P],
                                q_B, start=True, stop=True)
                            return ps

                        def do_exp(ps):
                            pT = ptp.tile([P, 2, NQ], BF16)
                            nc.scalar.activation(pT[:], ps[:],
                                                 mybir.ActivationFunctionType.Exp,
                                                 scale=SCALE)
                            return pT

                        def av(kc, pT):
                            nc.tensor.matmul(
                                poA[:], vA[:, kc, gA, :], pT[:, 0, :],
                                start=(kc == 0), stop=(kc == KC - 1))
                            nc.tensor.matmul(
                                poB[:], vA[:, kc, gB, :], pT[:, 1, :],
                                start=(kc == 0), stop=(kc == KC - 1))

                        def filler(kc):
                            # even-pair positions 0..3: four V-high columns
                            # per pair (two computed at each even kc, copied
                            # out at the following odd kc)
                            if pi < 4 and kc < 4:
                                if kc % 2 == 0:
                                    fl = flp.tile([P, 512], F32, tag="fl")
                                    state["fl"] = fl
                                    vv = pi * 4 + kc
                                    for tt in range(CT):
                                        nc.tensor.matmul(
                                            fl[:, 0:P],
                                            xbTs[:, tt, vv * P:(vv + 1) * P],
                                            wv[:, tt, P:DG],
                                            start=(tt == 0), stop=False)
                                    nc.tensor.matmul(
                                        fl[:, 0:P], ones1[0:1, 0:P],
                                        bvr[0:1, P:DG],
                                        start=False, stop=True)
                                    nc.tensor.matmul(
                                        fl[:, P:2 * P], ones1[0:1, 0:P],
                                        bvr[0:1, P:DG],
                                        start=True, stop=False)
                                    vv2 = pi * 4 + kc + 1
                                    for tt in range(CT):
                                        nc.tensor.matmul(
                                            fl[:, P:2 * P],
                                            xbTs[:, tt, vv2 * P:(vv2 + 1) * P],
                                            wv[:, tt, P:DG],
                                            start=False, stop=(tt == CT - 1))
                                else:
                                    fl = state["fl"]
                                    vv = pi * 4 + kc - 1
                                    with nc.allow_low_precision(
                                            reason="bf16 v staging"):
                                        nc.vector.tensor_copy(
                                            vA[:, vv, 2:4, 0:HD],
                                            fl[:, 0:P].rearrange(
                                                "p (g d) -> p g d", g=2))
                                        nc.vector.tensor_copy(
                                            vA[:, vv + 1, 2:4, 0:HD],
                                            fl[:, P:2 * P].rearrange(
                                                "p (g d) -> p g d", g=2))
                            # next pair's Q^T chunk, one matmul per kc 4..11
                            if pi + 1 < len(pair_order) and 4 <= kc < 12:
                                tn = pair_order[pi + 1]
                                tt = kc - 4
                                if kc == 4:
                                    state["fl"] = flp.tile([P, 512], F32,
                                                           tag="fl", name="flq")
                                fl = state["fl"]
                                nc.tensor.matmul(
                                    fl[:], wq[:, tt, tn * P:(tn + 1) * P],
                                    xbTs[:, tt, 0:NQ],
                                    start=(kc == 4), stop=False)
                                if kc == 11:
                                    nc.tensor.matmul(
                                        fl[:], bqr[0:1, tn * P:(tn + 1) * P],
                                        ones1[0:1, :], start=False, stop=True)
                                    with nc.allow_low_precision(
                                            reason="bf16 q staging"):
                                        nc.vector.tensor_copy(qT[:, tn, :],
                                                              fl[:])

                        ps_cur = scores(0)
                        for kc in range(KC):
                            pT = do_exp(ps_cur)
                            if kc + 1 < KC:
                                ps_cur = scores(kc + 1)
                            filler(kc)
                            av(kc, pT)

                        with nc.allow_low_precision(reason="bf16 attention staging"):
                            nc.vector.tensor_copy(oT[0:HD, t, :], poA[0:HD, :])
                            nc.vector.tensor_copy(oT[HD:P, t, :], poB[0:HD, :])
                            rd = rdp.tile([1, 3, 2, NQ], F32, tag="rd")
                            rdb = rdp.tile([1, 2, NQ], BF16, tag="rdb")
                            nc.vector.tensor_copy(rd[0:1, 0, 0, :], poA[HD:HD + 1, :])
                            nc.vector.tensor_copy(rd[0:1, 0, 1, :], poB[HD:HD + 1, :])
                            nc.vector.reciprocal_approx_accurate(
                                rd[0:1, 1, :, :], rd[0:1, 0, :, :], rd[0:1, 2, :, :])
                            nc.vector.tensor_copy(rdb[:], rd[0:1, 1, :, :])
                            pb = pbp.tile([P, NQ], F32, tag="pb")
                            nc.tensor.matmul(pb[:], eh[0:1, 0, :], rdb[0:1, 0, :],
                                             start=True, stop=False)
                            nc.tensor.matmul(pb[:], eh[0:1, 1, :], rdb[0:1, 1, :],
                                             start=False, stop=True)
                            nc.vector.tensor_mul(oT[:, t, :], oT[:, t, :], pb[:])

            # -------- out-proj --------
            # contraction over t in pair-completion order so the first
            # matmuls never wait on the last pairs' normalize (keeps the
            # PE busy through the transition, avoiding a HAM re-throttle)
            with tc.tile_pool(name="py", bufs=2, space="PSUM") as pyp, \
                 tc.tile_pool(name="ysb", bufs=2) as ysb:
                for m in range(NQ // P):
                    for fh in range(C // 512):
                        py = pyp.tile([P, 512], F32, tag="py")
                        for ti, t in enumerate(pair_order):
                            nc.tensor.matmul(
                                py[:], oT[:, t, m * P:(m + 1) * P],
                                wo[:, t, fh * 512:(fh + 1) * 512],
                                start=(ti == 0), stop=False)
                        nc.tensor.matmul(py[:], ones1[0:1, 0:P],
                                         bor[0:1, fh * 512:(fh + 1) * 512],
                                         start=False, stop=True)
                        yt = ysb.tile([P, 512], F32)
                        nc.vector.tensor_copy(yt[:], py[:])
                        nc.sync.dma_start(
                            out=y[m * P:(m + 1) * P, fh * 512:(fh + 1) * 512],
                            in_=yt[:])

    nc.compile()
    return nc


def _get_nc():
    if "nc" not in _CACHE:
        _CACHE["nc"] = _build()
    return _CACHE["nc"]


LAST_RESULTS = None


def kernel(x, Wq, bq, Wk, bk, Wv, bv, Wo, bo, trace=False, **trace_kwargs):
    x = np.asarray(x, dtype=np.float32)
    WqB = np.ascontiguousarray(np.asarray(Wq, dtype=np.float32).astype(NPBF))
    WkB = np.ascontiguousarray(np.asarray(Wk, dtype=np.float32).astype(NPBF))
    WvB = np.ascontiguousarray(np.asarray(Wv, dtype=np.float32).astype(NPBF))
    WoB = np.ascontiguousarray(np.asarray(Wo, dtype=np.float32).astype(NPBF))
    bqF = np.ascontiguousarray(np.asarray(bq, dtype=np.float32).astype(NPBF))
    bkF = np.ascontiguousarray(np.asarray(bk, dtype=np.float32).astype(NPBF))
    bvB = np.ascontiguousarray(np.asarray(bv, dtype=np.float32).astype(NPBF))
    boB = np.ascontiguousarray(np.asarray(bo, dtype=np.float32).astype(NPBF))

    nc = _get_nc()
    in_maps = []
    for d in range(NCORES):
        b, j = d // SPLIT, d % SPLIT
        # Roll the key/token axis so this core's queries are columns 0:NQ.
        # Attention is permutation-invariant over keys, so K/V built from the
        # rolled order give identical outputs.
        xbTr = np.ascontiguousarray(
            np.roll(x[b].T, -j * NQ, axis=1).astype(NPBF))
        in_maps.append({
            "xbT": xbTr,
            "Wq": WqB, "Wk": WkB, "Wv": WvB, "Wo": WoB,
            "bq": bqF, "bk": bkF, "bv": bvB, "bo": boB,
        })

    res = run_bass_kernel_spmd(nc, in_maps, core_ids=list(range(NCORES)),
                               trace=trace, **trace_kwargs)
    global LAST_RESULTS
    LAST_RESULTS = res

    out = np.empty((B, N, C), dtype=np.float32)
    for d in range(NCORES):
        b, j = d // SPLIT, d % SPLIT
        out[b, j * NQ:(j + 1) * NQ, :] = res.results[d]["y"]
    return out


# revision 25
# speedup vs baseline: 1.0931x; 1.0931x over previous
"""GroupedQueryAttention Trainium2 kernel (bf16 pipeline).

Problem shapes (hardcoded): x [2, 2048, 1024], H=16 heads, G=4 kv-groups,
head_dim=64.  out = softmax((xWq)(xWk)^T / 8) (xWv) Wo + biases.

Sharding: 8 cores, core d = (b, j) with b = d // 4, j = d % 4.
Each core computes the full attention output for batch b, query rows
[512j, 512j+512), all 16 heads — output rows are complete per core, so the
host-side gather is a pure concat (no reduction).
K/V are computed per-core for the whole batch (cheap 4x duplication).
The token axis of x^T is rolled per-core so queries are always columns
0:512 (attention is permutation-invariant over keys), keeping the SPMD
program identical across cores.

All matmul operands are bf16 (host-cast), PSUM accumulation fp32.  The
f32r path used previously lowers to fp32_mode=HIGH matmuls on HW (~3x
slower) and defeats fast-weight-load; bf16 fixes both and halves DMA.

On-chip dataflow (per core), "feature-on-partition" layout:
  x^T is pre-transposed + bf16-cast on host and DMA'd directly.
  K^T[dg,n]  = Wk^T x_b^T   (PSUM accum over c-chunks, + bias on DVE)
  V[n,dg]    = x_b Wv       (natural layout, + ones column for softmax denom)
  Q^T[d,nq]  = Wq^T x_q^T
  Attention runs per head-PAIR (2t, 2t+1) — both heads of a pair share
  Q^T chunk t (rows 0:64 / 64:128) and land in oT chunk t; their kv
  groups (2t%4, (2t+1)%4) share a kT gt-chunk.  Per k-chunk:
    S^T[k, 2, nq] = K Q^T   (two matmuls into one PSUM tile)
    P^T = exp(S^T / 8)      (one ScalarE activation per 1024-wide batch,
                             scale folded; logits are O(1), no max sub)
    O^T[65,nq] += [V|1]^T P^T  (per head; row 64 = denominator)
  Program order emits exp(kc); scores(kc+1); AV(kc) so the in-order
  TensorE queue hides AV's wait-for-exp behind next-chunk scores.
  normalize: per pair, reciprocal_approx_fast on the two denom rows,
  broadcast to 128 partitions via a K=2 one-hot matmul, one DVE mul.
  y[nq, c]   = O^T^T Wo     (accumulate over c-chunks) + bo
"""

import ml_dtypes
import numpy as np

import concourse.bacc as bacc
import concourse.mybir as mybir
import concourse.tile as tile
from concourse.bass_utils import run_bass_kernel_spmd

# ---- problem constants (hardcoded per contract) ----
B, N, C = 2, 2048, 1024
H, G, HD = 16, 4, 64
DG = G * HD            # 256
NCORES = 8
SPLIT = NCORES // B    # 4 query splits per batch
NQ = N // SPLIT        # 512 query rows per core
P = 128
CT = C // P            # 8 c-chunks
KC = N // P            # 16 k-chunks
SCALE = HD ** -0.5

F32 = mybir.dt.float32
BF16 = mybir.dt.bfloat16
NPBF = ml_dtypes.bfloat16

_CACHE = {}


def _build():
    nc = bacc.Bacc(None, target_bir_lowering=False)

    xbT = nc.declare_dram_parameter("xbT", [C, N], BF16, isOutput=False)
    Wq = nc.declare_dram_parameter("Wq", [C, C], BF16, isOutput=False)
    Wk = nc.declare_dram_parameter("Wk", [C, DG], BF16, isOutput=False)
    Wv = nc.declare_dram_parameter("Wv", [C, DG], BF16, isOutput=False)
    Wo = nc.declare_dram_parameter("Wo", [C, C], BF16, isOutput=False)
    bq = nc.declare_dram_parameter("bq", [C], BF16, isOutput=False)
    bk = nc.declare_dram_parameter("bk", [DG], BF16, isOutput=False)
    bv = nc.declare_dram_parameter("bv", [DG], BF16, isOutput=False)
    bo = nc.declare_dram_parameter("bo", [C], BF16, isOutput=False)
    y = nc.declare_dram_parameter("y", [NQ, C], F32, isOutput=True)

    with tile.TileContext(nc) as tc:
        with tc.tile_pool(name="main", bufs=1) as main:
            qT = main.tile([P, CT, NQ], BF16)         # Q^T  d-chunk x q
            kT = main.tile([P, 2, N], BF16)           # K^T  dg-chunk x k
            vA = main.tile([P, KC, G, HD + 1], BF16)  # V + ones col, per k-chunk
            oT = main.tile([P, CT, NQ], BF16)         # O^T (unnorm -> normed)
            wo = main.tile([P, CT, C], BF16)
            eh = main.tile([1, 2, P], BF16)           # one-hot head->rows map
            ones1 = main.tile([1, 512], BF16)         # ones row (lhs or rhs)
            bqr = main.tile([1, C], BF16)             # bias rows (natural)
            bkr = main.tile([1, DG], BF16)
            bvr = main.tile([1, DG], BF16)
            bor = main.tile([1, C], BF16)

            # ---------------- phase A+B: load + projections ----------------
            # All DMAs on the sync queue in strict priority order: xbT+wk
            # first (K streams against their arrival), then the small
            # constants, then wv, wq, wo.  One queue keeps HBM bandwidth
            # focused on the earliest-needed tensors.
            with tc.tile_pool(name="proj", bufs=1) as proj:
                xbTs = proj.tile([P, CT, N], BF16)
                wq = proj.tile([P, CT, C], BF16)
                wk = proj.tile([P, CT, DG], BF16)
                wv = proj.tile([P, CT, DG], BF16)
                vTs = proj.tile([P, 2, N], BF16)      # V^T staging
                ident = proj.tile([P, P], BF16)
                for t in range(CT):
                    nc.sync.dma_start(out=xbTs[:, t, :], in_=xbT[t * P:(t + 1) * P, :])
                    nc.sync.dma_start(out=wk[:, t, :], in_=Wk[t * P:(t + 1) * P, :])
                # constants DMA'd from NEFF-embedded data
                e_np = np.zeros((1, 2, P), NPBF)
                e_np[0, 0, 0:HD] = 1.0   # even head of pair -> rows 0..63
                e_np[0, 1, HD:P] = 1.0   # odd head of pair -> rows 64..127
                nc.sync.dma_start(out=eh[:], in_=nc.inline_tensor(e_np, "ehot")[:])
                nc.sync.dma_start(
                    out=ones1[:],
                    in_=nc.inline_tensor(np.ones((1, 512), NPBF), "ones1")[:])
                nc.sync.dma_start(
                    out=ident[:],
                    in_=nc.inline_tensor(np.eye(P, dtype=NPBF), "ident")[:])
                vcol_np = np.ones((P, KC * G), NPBF)
                nc.sync.dma_start(
                    out=vA[:, :, :, HD:HD + 1],
                    in_=nc.inline_tensor(vcol_np, "vcol")[:]
                    .rearrange("p (k g o) -> p k g o", g=G, o=1))
                nc.sync.dma_start(out=bqr[:], in_=bq.rearrange("(o d) -> o d", o=1))
                nc.sync.dma_start(out=bkr[:], in_=bk.rearrange("(o d) -> o d", o=1))
                nc.sync.dma_start(out=bvr[:], in_=bv.rearrange("(o d) -> o d", o=1))
                nc.sync.dma_start(out=bor[:], in_=bo.rearrange("(o d) -> o d", o=1))
                for t in range(CT):
                    nc.sync.dma_start(out=wv[:, t, :], in_=Wv[t * P:(t + 1) * P, :])
                for t in range(CT):
                    nc.sync.dma_start(out=wq[:, t, :], in_=Wq[t * P:(t + 1) * P, :])
                for t in range(CT):
                    nc.sync.dma_start(out=wo[:, t, :], in_=Wo[t * P:(t + 1) * P, :])

                # pre-warm the exp table set while DMAs stream
                warm = proj.tile([1, 2], F32)
                nc.scalar.activation(warm[:], ones1[0:1, 0:2],
                                     mybir.ActivationFunctionType.Exp)

                with nc.allow_low_precision(reason="bf16 staging of projections"):
                    # K^T, t-streamed against DMA arrival: all 8 output
                    # blocks accumulate in parallel (8 PSUM banks) so the
                    # first matmul only needs chunk 0 of xbT/wk.  Bias is
                    # added via a [1,w] x [1,512] broadcast matmul (bias
                    # values as weights, ones as the moving operand).
                    with tc.tile_pool(name="pk8", bufs=1, space="PSUM") as pk8:
                        pks = [pk8.tile([P, 512], F32, tag=f"pk{b}",
                                        name=f"pk{b}")
                               for b in range(8)]
                        for t in range(CT):
                            for gt in range(2):
                                for nf in range(N // 512):
                                    nc.tensor.matmul(
                                        pks[gt * 4 + nf],
                                        wk[:, t, gt * P:(gt + 1) * P],
                                        xbTs[:, t, nf * 512:(nf + 1) * 512],
                                        start=(t == 0), stop=False)
                        for gt in range(2):
                            for nf in range(N // 512):
                                nc.tensor.matmul(
                                    pks[gt * 4 + nf],
                                    bkr[0:1, gt * P:(gt + 1) * P],
                                    ones1[0:1, :], start=False, stop=True)
                                nc.vector.tensor_copy(
                                    kT[:, gt, nf * 512:(nf + 1) * 512],
                                    pks[gt * 4 + nf])

                    # V^T computed like K (16 ldweights instead of 128),
                    # then each [128,128] chunk is transposed back through
                    # the PE array into V-natural layout for AV matmuls.
                    with tc.tile_pool(name="pp", bufs=2, space="PSUM") as pp:
                        for dgc in range(2):
                            for nf in range(N // 512):
                                pv = pp.tile([P, 512], F32, tag="pv")
                                for t in range(CT):
                                    nc.tensor.matmul(
                                        pv[:], wv[:, t, dgc * P:(dgc + 1) * P],
                                        xbTs[:, t, nf * 512:(nf + 1) * 512],
                                        start=(t == 0), stop=False)
                                nc.tensor.matmul(
                                    pv[:], bvr[0:1, dgc * P:(dgc + 1) * P],
                                    ones1[0:1, :], start=False, stop=True)
                                nc.vector.tensor_copy(
                                    vTs[:, dgc, nf * 512:(nf + 1) * 512],
                                    pv[:])
                                for kc in range(nf * 4, nf * 4 + 4):
                                    ptr = pp.tile([P, P], BF16, tag="ptr")
                                    nc.tensor.transpose(
                                        ptr[:],
                                        vTs[:, dgc, kc * P:(kc + 1) * P],
                                        ident[:])
                                    nc.vector.tensor_copy(
                                        vA[:, kc, 2 * dgc:2 * dgc + 2, 0:HD],
                                        ptr[:].rearrange("p (g d) -> p g d",
                                                         g=2))

                        # Q^T chunk 0 only; chunks 1..7 run as attention filler
                        pq = pp.tile([P, NQ], F32, tag="pv")
                        for t in range(CT):
                            nc.tensor.matmul(
                                pq[:], wq[:, t, 0:P],
                                xbTs[:, t, 0:NQ], start=(t == 0), stop=False)
                        nc.tensor.matmul(pq[:], bqr[0:1, 0:P], ones1[0:1, :],
                                         start=False, stop=True)
                        nc.vector.tensor_copy(qT[:, 0, :], pq[:])

                # ---- phase C: attention (per head-pair) with fillers ----
                # The next pair's Q^T chunk is emitted as TensorE filler
                # inside each ScalarE-bound kc loop.
                pair_order = list(range(CT))
                with tc.tile_pool(name="pt", bufs=3) as ptp, \
                     tc.tile_pool(name="rd", bufs=2) as rdp, \
                     tc.tile_pool(name="ps", bufs=2, space="PSUM") as psp, \
                     tc.tile_pool(name="po", bufs=1, space="PSUM") as pop, \
                     tc.tile_pool(name="pb", bufs=1, space="PSUM") as pbp, \
                     tc.tile_pool(name="fl", bufs=1, space="PSUM") as flp:
                    for pi, t in enumerate(pair_order):
                        hA, hB = 2 * t, 2 * t + 1
                        gA, gB = hA % G, hB % G
                        gtA, grA = gA // 2, (gA % 2) * HD
                        gtB, grB = gB // 2, (gB % 2) * HD
                        q_A = qT[0:HD, t, :]
                        q_B = qT[HD:P, t, :]
                        poA = pop.tile([HD + 1, NQ], F32, tag="poA")
                        poB = pop.tile([HD + 1, NQ], F32, tag="poB")
                        state = {"fl": None}

                        def scores(kc):
                            ps = psp.tile([P, 2, NQ], F32)
                            nc.tensor.matmul(
                                ps[:, 0, :],
                                kT[grA:grA + HD, gtA, kc * P:(kc + 1) * P],
                                q_A, start=True, stop=True)
                            nc.tensor.matmul(
                                ps[:, 1, :],
                                kT[grB:grB + HD, gtB, kc * P:(kc + 1) * P],
                                q_B, start=True, stop=True)
                            return ps

                        def do_exp(ps):
                            pT = ptp.tile([P, 2, NQ], BF16)
                            nc.scalar.activation(pT[:], ps[:],
                                                 mybir.ActivationFunctionType.Exp,
                                                 scale=SCALE)
                            return pT

                        def av(kc, pT):
                            nc.tensor.matmul(
                                poA[:], vA[:, kc, gA, :], pT[:, 0, :],
                                start=(kc == 0), stop=(kc == KC - 1))
                            nc.tensor.matmul(
                                poB[:], vA[:, kc, gB, :], pT[:, 1, :],
                                start=(kc == 0), stop=(kc == KC - 1))

                        def filler(kc):
                            # next pair's Q^T chunk, one matmul per kc 4..11
                            if pi + 1 < len(pair_order) and 4 <= kc < 12:
                                tn = pair_order[pi + 1]
                                tt = kc - 4
                                if kc == 4:
                                    state["fl"] = flp.tile([P, 512], F32,
                                                           tag="fl", name="flq")
                                fl = state["fl"]
                                nc.tensor.matmul(
                                    fl[:], wq[:, tt, tn * P:(tn + 1) * P],
                                    xbTs[:, tt, 0:NQ],
                                    start=(kc == 4), stop=False)
                                if kc == 11:
                                    nc.tensor.matmul(
                                        fl[:], bqr[0:1, tn * P:(tn + 1) * P],
                                        ones1[0:1, :], start=False, stop=True)
                                    with nc.allow_low_precision(
                                            reason="bf16 q staging"):
                                        nc.vector.tensor_copy(qT[:, tn, :],
                                                              fl[:])

                        ps_cur = scores(0)
                        for kc in range(KC):
                            pT = do_exp(ps_cur)
                            if kc + 1 < KC:
                                ps_cur = scores(kc + 1)
                            filler(kc)
                            av(kc, pT)

                        with nc.allow_low_precision(reason="bf16 attention staging"):
                            nc.vector.tensor_copy(oT[0:HD, t, :], poA[0:HD, :])
                            nc.vector.tensor_copy(oT[HD:P, t, :], poB[0:HD, :])
                            rd = rdp.tile([1, 3, 2, NQ], F32, tag="rd")
                            rdb = rdp.tile([1, 2, NQ], BF16, tag="rdb")
                            nc.vector.tensor_copy(rd[0:1, 0, 0, :], poA[HD:HD + 1, :])
                            nc.vector.tensor_copy(rd[0:1, 0, 1, :], poB[HD:HD + 1, :])
                            nc.vector.reciprocal_approx_accurate(
                                rd[0:1, 1, :, :], rd[0:1, 0, :, :], rd[0:1, 2, :, :])
                            nc.vector.tensor_copy(rdb[:], rd[0:1, 1, :, :])
                            pb = pbp.tile([P, NQ], F32, tag="pb")
                            nc.tensor.matmul(pb[:], eh[0:1, 0, :], rdb[0:1, 0, :],
                                             start=True, stop=False)
                            nc.tensor.matmul(pb[:], eh[0:1, 1, :], rdb[0:1, 1, :],
                                             start=False, stop=True)
                            nc.vector.tensor_mul(oT[:, t, :], oT[:, t, :], pb[:])

            # -------- out-proj --------
            # contraction over t in pair-completion order so the first
            # matmuls never wait on the last pairs' normalize (keeps the
            # PE busy through the transition, avoiding a HAM re-throttle)
            with tc.tile_pool(name="py", bufs=2, space="PSUM") as pyp, \
                 tc.tile_pool(name="ysb", bufs=2) as ysb:
                for m in range(NQ // P):
                    for fh in range(C // 512):
                        py = pyp.tile([P, 512], F32, tag="py")
                        for ti, t in enumerate(pair_order):
                            nc.tensor.matmul(
                                py[:], oT[:, t, m * P:(m + 1) * P],
                                wo[:, t, fh * 512:(fh + 1) * 512],
                                start=(ti == 0), stop=False)
                        nc.tensor.matmul(py[:], ones1[0:1, 0:P],
                                         bor[0:1, fh * 512:(fh + 1) * 512],
                                         start=False, stop=True)
                        yt = ysb.tile([P, 512], F32)
                        nc.vector.tensor_copy(yt[:], py[:])
                        nc.sync.dma_start(
                            out=y[m * P:(m + 1) * P, fh * 512:(fh + 1) * 512],
                            in_=yt[:])

    nc.compile()
    return nc


def _get_nc():
    if "nc" not in _CACHE:
        _CACHE["nc"] = _build()
    return _CACHE["nc"]


LAST_RESULTS = None


def kernel(x, Wq, bq, Wk, bk, Wv, bv, Wo, bo, trace=False, **trace_kwargs):
    x = np.asarray(x, dtype=np.float32)
    WqB = np.ascontiguousarray(np.asarray(Wq, dtype=np.float32).astype(NPBF))
    WkB = np.ascontiguousarray(np.asarray(Wk, dtype=np.float32).astype(NPBF))
    WvB = np.ascontiguousarray(np.asarray(Wv, dtype=np.float32).astype(NPBF))
    WoB = np.ascontiguousarray(np.asarray(Wo, dtype=np.float32).astype(NPBF))
    bqF = np.ascontiguousarray(np.asarray(bq, dtype=np.float32).astype(NPBF))
    bkF = np.ascontiguousarray(np.asarray(bk, dtype=np.float32).astype(NPBF))
    bvB = np.ascontiguousarray(np.asarray(bv, dtype=np.float32).astype(NPBF))
    boB = np.ascontiguousarray(np.asarray(bo, dtype=np.float32).astype(NPBF))

    nc = _get_nc()
    in_maps = []
    for d in range(NCORES):
        b, j = d // SPLIT, d % SPLIT
        # Roll the key/token axis so this core's queries are columns 0:NQ.
        # Attention is permutation-invariant over keys, so K/V built from the
        # rolled order give identical outputs.
        xbTr = np.ascontiguousarray(
            np.roll(x[b].T, -j * NQ, axis=1).astype(NPBF))
        in_maps.append({
            "xbT": xbTr,
            "Wq": WqB, "Wk": WkB, "Wv": WvB, "Wo": WoB,
            "bq": bqF, "bk": bkF, "bv": bvB, "bo": boB,
        })

    res = run_bass_kernel_spmd(nc, in_maps, core_ids=list(range(NCORES)),
                               trace=trace, **trace_kwargs)
    global LAST_RESULTS
    LAST_RESULTS = res

    out = np.empty((B, N, C), dtype=np.float32)
    for d in range(NCORES):
        b, j = d // SPLIT, d % SPLIT
        out[b, j * NQ:(j + 1) * NQ, :] = res.results[d]["y"]
    return out
